# revision 5
# baseline (speedup 1.0000x reference)
"""ConvTranspose3d (C_in=128, C_out=64, k=4, stride=2, pad=1) on 8 Trainium2
NeuronCores.

Strategy: polyphase decomposition. A stride-2 transposed conv splits into 8
output parity classes (od%2, oh%2, ow%2); each class is a stride-1 conv with
2x2x2 taps over the input grid. Sharding: core <-> (batch n, oh parity rh,
ow parity rw); the two od parities are computed together in one 128-partition
PSUM tile (partitions = rd*64 + cout), so each matmul contracts Cin=128 and
produces 128 outputs - full PE width.

SPMD wrinkle: all cores run one program, but (rh, rw) change which input
shifts each tap needs. Solved host-side: the 34x34 padded frame is shifted by
(1-rh, 1-rw) per core and the tap->weight assignment is permuted to match, so
the compiled AP offsets are identical across cores.

Matmuls run in float32r (1 PE cycle/row at N=512, ~1e-4 rel err). All
cross-engine deps funnel through the scalar engine (ACT): it converts
fp32->f32r and drains PSUM with the bias add, so every instruction needs at
most one sem wait - this toolchain's walrus rejects instructions with more.
"""
import numpy as np

import concourse.bass as bass
import concourse.mybir as mybir
import concourse.tile as tile
from concourse.bass_utils import run_bass_kernel_spmd

F32 = mybir.dt.float32
F32R = mybir.dt.float32r
ACT_COPY = mybir.ActivationFunctionType.Copy
IDENT = mybir.ActivationFunctionType.Identity

N_BATCH, C_IN, C_OUT = 2, 128, 64
N_D, N_HW = 16, 32  # input spatial grid
N_CORES = 8

# tap order: dgrp-major (d-shift 0,-1,+1), then h-option, then w-option.
# h/w options are (row-base 1, row-base 0); the host maps each base to the
# right kernel index per core parity.
_TAPS = [(dd, hb, wb) for dd in (0, -1, 1) for hb in (1, 0) for wb in (1, 0)]


class _SplitDrainTileContext(tile.TileContext):
    """TileContext whose kernel-tail drain is split into one drain per proc
    (this walrus build rejects instructions carrying more than ~2 sync
    commands, and the stock tail drain waits on every active proc)."""

    def _drain_and_barrier(self, tick_clock, wait_clock):
        from concourse.vector_clock import ScopedClock, VectorClock

        gc = tick_clock.global_clock
        n = len(gc)
        for i in range(n):
            t = gc[i]
            if t <= 0:
                continue
            vc = VectorClock([0] * n)
            vc.require_at_least(i, t)
            d = self.nc.sync.drain()
            wait_clock.add_sem_waits(d.ins, ScopedClock({None: vc}))
        self.nc.all_engine_barrier()
        assert self.sems is not None
        popped = self.nc._tile_sem_poison_stack.pop()
        assert popped is self._sem_poison
        self.nc.clear_and_free_semaphores(list(self.sems.allocated().values()))
        self.nc.all_engine_barrier()


def _build_program():
    nc = bass.Bass()
    # fp32 bytes are DMA'd straight into f32r tiles: the PE rounds f32r
    # operands on ingest (verified identical to an explicit ACT convert)
    xp_in = nc.declare_dram_parameter("xp", [C_IN, N_D, 34, 34], F32R, isOutput=False)
    wt_in = nc.declare_dram_parameter("wt", [C_IN, 12 * 128], F32R, isOutput=False)
    b_in = nc.declare_dram_parameter("b2", [128, 1], F32, isOutput=False)
    # one output tensor per round: a single interleaved output tensor makes
    # Tile's DRAM range tracker see round writes as overlapping and add
    # cross-queue WAW waits that bust the 1-wait DMA budget
    outs = [
        nc.declare_dram_parameter(f"out{r}", [128, 2, 1024], F32, isOutput=True)
        for r in range(N_D // 2)
    ]
    BF16 = mybir.dt.bfloat16

    with _SplitDrainTileContext(nc) as tc:
        with (
            tc.tile_pool(name="const", bufs=1) as cpool,
            tc.tile_pool(name="xslices", bufs=1) as xpool,
            tc.tile_pool(name="ps", bufs=2, space="PSUM") as pspool,
        ):
            lw = cpool.tile([128, 12 * 128], F32R)
            nc.sync.dma_start(lw[:], wt_in[:])
            # PE-engine observer: dummy bf16 ldweights reading each DMA'd
            # tile makes the PE observe the DMA tick, so matmuls never carry
            # a DMA wait on top of their ACT wait (1-wait budget)
            nc.tensor.ldweights(lw[:, 0:1].bitcast(BF16))

            br = cpool.tile([128, 1], F32)
            nc.sync.dma_start(br[:], b_in[:])
            bia = cpool.tile([128, 1], F32)
            nc.scalar.activation(bia[:], br[:], ACT_COPY)
            # absorb the ACT-pipeline self-wait on bia once, so drains below
            # only ever wait on PE
            obs = cpool.tile([128, 1], F32)
            nc.scalar.activation(obs[:], bia[:], ACT_COPY)
            out_sb = cpool.tile([128, N_D, N_HW, N_HW], F32)

            xd = []
            for i in range(N_D):
                xt = xpool.tile([128, 34, 34], F32R, name=f"xd_{i}", tag=f"xd{i}")
                nc.sync.dma_start(xt[:], xp_in[:, i])
                xd.append(xt)

            observed = set()

            def observe(i):
                if i not in observed:
                    nc.tensor.ldweights(xd[i][:, 0, 0:1].bitcast(BF16))
                    observed.add(i)

            for r in range(N_D // 2):
                ms = (2 * r, 2 * r + 1)
                for s in range(2 * r, min(2 * r + 3, N_D)):
                    observe(s)
                groups = [(m, h) for m in ms for h in (0, 1)]
                pst = {}
                for gi, g in enumerate(groups):
                    pst[g] = pspool.tile(
                        [128, 16, 32], F32, name=f"ps_{r}_{gi}", tag=f"ps{gi}"
                    )
                last_t = {
                    g: max(t for t in range(12) if 0 <= g[0] + _TAPS[t][0] < N_D)
                    for g in groups
                }
                started = {g: False for g in groups}
                for t in range(12):
                    dd, hb, wb = _TAPS[t]
                    for g in groups:
                        m, h = g
                        i = m + dd
                        if not (0 <= i < N_D):
                            continue
                        nc.tensor.matmul(
                            pst[g][:],
                            lw[:, t * 128 : (t + 1) * 128],
                            xd[i][:, hb + 16 * h : hb + 16 * h + 16, wb : wb + 32],
                            start=not started[g],
                            stop=(t == last_t[g]),
                        )
                        started[g] = True
                for m, h in groups:
                    nc.scalar.activation(
                        out_sb[:, m, 16 * h : 16 * h + 16, :],
                        pst[(m, h)][:],
                        IDENT,
                        bias=bia[:],
                    )
                # SWDGE (gpsimd) queues are otherwise unused, so this DMA
                # carries only its ACT data wait (1-wait budget)
                nc.gpsimd.dma_start(
                    outs[r][:],
                    out_sb[:, ms[0] : ms[1] + 1].rearrange("p c d e -> p c (d e)"),
                )
    return nc


_NC_CACHE = None


def _get_program():
    global _NC_CACHE
    if _NC_CACHE is None:
        _NC_CACHE = _build_program()
    return _NC_CACHE


def _k_of(parity, base):
    # kernel index along one spatial dim for the tap option with the given
    # padded-frame row base, for output parity `parity` (frame shift 1-parity)
    return {(0, 1): 1, (0, 0): 3, (1, 1): 0, (1, 0): 2}[(parity, base)]


def _build_w_stack(weight, rh, rw):
    """(128, 12*128) fp32: stacked lhsT per tap; cols 0:64 = rd=0, 64:128 = rd=1."""
    stack = np.zeros((C_IN, 12 * 128), np.float32)
    for t, (dd, hb, wb) in enumerate(_TAPS):
        kh = _k_of(rh, hb)
        kw = _k_of(rw, wb)
        L = stack[:, t * 128 : (t + 1) * 128]
        if dd == 0:
            L[:, 0:64] = weight[:, :, 1, kh, kw]
            L[:, 64:128] = weight[:, :, 2, kh, kw]
        elif dd == -1:
            L[:, 0:64] = weight[:, :, 3, kh, kw]
        else:
            L[:, 64:128] = weight[:, :, 0, kh, kw]
    return stack


def kernel(x, weight, bias):
    x = np.asarray(x, dtype=np.float32)
    weight = np.asarray(weight, dtype=np.float32)
    bias = np.asarray(bias, dtype=np.float32)
    nc = _get_program()

    bias2 = np.concatenate([bias, bias]).astype(np.float32).reshape(128, 1)
    in_maps = []
    for core in range(N_CORES):
        n, rh, rw = core // 4, (core // 2) % 2, core % 2
        sh, sw = 1 - rh, 1 - rw
        xp = np.zeros((C_IN, N_D, 34, 34), np.float32)
        xp[:, :, sh : sh + 32, sw : sw + 32] = x[n]
        in_maps.append(
            {"xp": xp, "wt": _build_w_stack(weight, rh, rw), "b2": bias2}
        )

    res = run_bass_kernel_spmd(nc, in_maps, list(range(N_CORES)))

    y = np.empty((N_BATCH, C_OUT, 2 * N_D, 2 * N_HW, 2 * N_HW), np.float32)
    o = np.empty((2, C_OUT, N_D, N_HW, N_HW), np.float32)
    for core in range(N_CORES):
        n, rh, rw = core // 4, (core // 2) % 2, core % 2
        rr = res.results[core]
        for r in range(N_D // 2):
            chunk = rr[f"out{r}"].reshape(2, C_OUT, 2, N_HW, N_HW)
            o[:, :, 2 * r : 2 * r + 2] = chunk
        for rd in range(2):
            y[n, :, rd::2, rh::2, rw::2] = o[rd]
    return y
